# revision 1
# baseline (speedup 1.0000x reference)
"""Causal multi-head attention (B=4, S=2048, D=1024, H=16) on 8 TRN2 cores.

Sharding: core c -> (batch b = c//2, head-group g = c%2, 8 heads each).
Host pre-transposes/splits inputs; device returns per-core partial outputs
y_c = attn_heads(g) @ wo[g-rows]; host sums the two partials per batch.

Precision: q/k projections and QK^T run in single-pass float32r matmuls
(fp32 operands, ~12-bit internal mantissa, full rate at output>=256 rows).
QK^T packs K=128 as [q;q]x[k;k] (computes 2*q.k, folded into softmax scale
1/16), which runs ~2x faster than K=64 on HW. The causal mask, 1/16 scale,
negation, and row-max all fuse into DVE tensor_tensor_reduce passes; exp
runs on ScalarE with scale=-1 and bias=-max. Value path fp16 end-to-end.
Predicted end-to-end rel err ~7e-3 (host-sim with HW-calibrated noise).
"""

import numpy as np

import concourse.bacc as bacc
import concourse.tile as tile
from concourse import mybir
from concourse.bass_utils import run_bass_kernel_spmd

B, S, D = 4, 2048, 1024
H, DK = 16, 64
HL = 8            # heads per core
DL = HL * DK      # 512 local channels
N_CORES = 8
P = 128           # partitions
KT = D // P       # 8 contraction tiles
QT = S // P       # 16 q tiles
MS = 256          # proj m-slab (seq cols per x stage tile)
MT = S // MS      # 8
NT = DL // P      # 4 channel slabs of 128
CHUNK = 1024      # score chunk (2 PSUM banks)
BIG = 1.0e30

f32 = mybir.dt.float32
f32r = mybir.dt.float32r
f16 = mybir.dt.float16
ALU = mybir.AluOpType
AF = mybir.ActivationFunctionType
AX = mybir.AxisListType.X

_cache = {}


def _build():
    nc = bacc.Bacc("TRN2", target_bir_lowering=False)

    def din(name, shape, dt):
        return nc.dram_tensor(name, shape, dt, kind="ExternalInput").ap()

    xq = din("xq", [D, S], f32r)
    xk = din("xk", [D, S], f32r)
    xv = din("xv", [D, S], f16)
    wq = din("wq", [D, DL], f32r)
    wk = din("wk", [D, DL], f32r)
    wv = din("wv", [D, DL], f16)
    wo = din("wo", [DL, D], f16)
    ident = din("ident", [P, P], f32r)
    maskw = din("maskw", [P, 512], f32r)
    y = nc.dram_tensor("y", [S, D], f32, kind="ExternalOutput").ap()

    with tile.TileContext(nc) as tc:
        _body(nc, tc, xq, xk, xv, wq, wk, wv, wo, ident, maskw, y)
    nc.compile()
    return nc


def _body(nc, tc, xq, xk, xv, wq, wk, wv, wo, ident, maskw, y):
    from contextlib import ExitStack
    ctx = ExitStack()
    with ctx:
        # ---------- long-lived tiles ----------
        persist = ctx.enter_context(tc.tile_pool(name="persist", bufs=1))
        # qsb[n]: heads 2n (p0:64) and 2n+1 (p64:128), channel-major,
        # q pre-scaled by 0.125 so QK^T PSUM holds final scaled scores.
        # kdup[h]: [k_h; 0] for even h, [0; k_h] for odd h -- the zero half
        # annihilates the other head's q in qsb, so lhsT = full qsb tile
        # gives a K=128 full-rate matmul with no q duplication.
        qsb = [persist.tile([P, S], f32r, tag=f"qs_{n}", name=f"qs_{n}") for n in range(NT)]
        kdup = [persist.tile([P, S], f32r, tag=f"kd_{h}", name=f"kd_{h}") for h in range(HL)]
        for h in range(HL):
            zslice = kdup[h][DK:P, :] if h % 2 == 0 else kdup[h][0:DK, :]
            nc.vector.memset(zslice.bitcast(f32), 0.0)
        # vsb: [kpos, head, dk+1]; last column is ones so PV also yields Z
        vsb = [persist.tile([P, HL, DK + 1], f16, tag=f"v_{m}", name=f"v_{m}")
               for m in range(QT)]
        outT = persist.tile([P, NT, S], f16, tag="outT", name="outT")
        ident_sb = persist.tile([P, P], f32r, tag="ident")
        maskw_sb = persist.tile([P, 512], f32r, tag="maskw")
        nc.sync.dma_start(out=ident_sb, in_=ident)
        nc.sync.dma_start(out=maskw_sb, in_=maskw)

        # ---------- phase 1: projections ----------
        with (
            tc.tile_pool(name="wpool", bufs=1) as wpool,
            tc.tile_pool(name="xpool", bufs=2) as xpool,
            tc.tile_pool(name="ppsum", bufs=4, space="PSUM") as ppsum,
        ):
            wv_sb = wpool.tile([P, KT, DL], f16, tag="wv", name="wv")
            nc.sync.dma_start(out=wv_sb, in_=wv.rearrange("(k p) n -> p k n", p=P))
            wq_sb = wpool.tile([P, KT, DL], f32r, tag="wq", name="wq")
            nc.sync.dma_start(out=wq_sb, in_=wq.rearrange("(k p) n -> p k n", p=P))
            wk_sb = wpool.tile([P, KT, DL], f32r, tag="wk", name="wk")
            nc.sync.dma_start(out=wk_sb, in_=wk.rearrange("(k p) n -> p k n", p=P))

            # Q/K projections, channel-major out [DL, S] in m-slabs.
            # x staged in half-KT tiles to fit SBUF while keeping ap=256.
            KH = KT // 2
            for m in range(MT):
                msl = slice(m * MS, (m + 1) * MS)
                xq_h = [xpool.tile([P, KH, MS], f32r, tag="xq", name=f"xq_{half}")
                        for half in range(2)]
                xk_h = [xpool.tile([P, KH, MS], f32r, tag="xk", name=f"xk_{half}")
                        for half in range(2)]
                for half in range(2):
                    ksl = slice(half * KH, (half + 1) * KH)
                    nc.gpsimd.dma_start(
                        out=xq_h[half], in_=xq.rearrange("(k p) s -> p k s", p=P)[:, ksl, msl])
                    nc.gpsimd.dma_start(
                        out=xk_h[half], in_=xk.rearrange("(k p) s -> p k s", p=P)[:, ksl, msl])
                for n in range(NT):
                    csl = slice(n * P, (n + 1) * P)
                    psq = ppsum.tile([P, MS], f32, tag="proj")
                    for k in range(KT):
                        nc.tensor.matmul(psq[:], wq_sb[:, k, csl], xq_h[k // KH][:, k % KH],
                                         start=(k == 0), stop=(k == KT - 1))
                    # pre-scale q by 1/16 so QK^T PSUM holds final scaled scores
                    nc.vector.tensor_scalar_mul(qsb[n][:, msl], psq[:], 0.125)
                    psk = ppsum.tile([P, MS], f32, tag="proj")
                    for k in range(KT):
                        nc.tensor.matmul(psk[:], wk_sb[:, k, csl], xk_h[k // KH][:, k % KH],
                                         start=(k == 0), stop=(k == KT - 1))
                    nc.scalar.copy(kdup[2 * n][0:DK, msl], psk[0:DK, :])
                    nc.scalar.copy(kdup[2 * n + 1][DK:P, msl], psk[DK:P, :])

            # V projection -> seq-major [S, (h, dk+1)], fp16
            for m in range(QT):
                nc.vector.memset(vsb[m][:, :, DK:DK + 1], 1.0)
                xvt = xpool.tile([P, KT, P], f16, tag="xv", name="xvt")
                nc.sync.dma_start(
                    out=xvt, in_=xv.rearrange("(k p) s -> p k s", p=P)[:, :, m * P:(m + 1) * P])
                ps = ppsum.tile([P, DL], f32, tag="proj")
                for k in range(KT):
                    nc.tensor.matmul(ps[:], xvt[:, k], wv_sb[:, k],
                                     start=(k == 0), stop=(k == KT - 1))
                nc.scalar.copy(vsb[m][:, :, 0:DK], ps[:].rearrange("p (h d) -> p h d", h=HL))


        # ---------- phase 3: attention ----------
        with (
            tc.tile_pool(name="scpool", bufs=3, space="PSUM") as scpool,
            tc.tile_pool(name="pvpool", bufs=2, space="PSUM") as pvpool,
            tc.tile_pool(name="ppool", bufs=7) as ppool,
            tc.tile_pool(name="ptpool", bufs=7) as ptpool,
            tc.tile_pool(name="stat", bufs=8) as stat,
            tc.tile_pool(name="ostage", bufs=3) as ostage,
            tc.tile_pool(name="wopool", bufs=1) as wopool,
            tc.tile_pool(name="ypool", bufs=3) as ypool,
        ):
            wo_sb = wopool.tile([P, NT, D], f16, tag="wo")
            nc.sync.dma_start(out=wo_sb, in_=wo.rearrange("(j p) n -> p j n", p=P))
            # pipeline units = score chunks (uniform PSUM footprint so the
            # sc-slot reuse distance is always `bufs` units, keeping the
            # exp->qk WAR edge 3 units deep to hide semaphore latency)
            units = []
            for qt in range(QT):
                for h in range(HL):
                    klen = (qt + 1) * P
                    chs = [(0, klen)] if klen <= CHUNK else [(0, CHUNK), (CHUNK, klen)]
                    for ci, (c0, c1) in enumerate(chs):
                        units.append((qt, h, ci, c0, c1, len(chs)))
            state = {}
            hstate = {}

            def get_qt_tiles(qt):
                if qt not in state:
                    state[qt] = dict(
                        ostg=ostage.tile([P, DL], f16, tag="ostg", name="ostg"),
                        pt={})
                return state[qt]

            def chunks_of(qt):
                klen = (qt + 1) * P
                return [(0, klen)] if klen <= CHUNK else [(0, CHUNK), (CHUNK, klen)]

            def alpha_chunk(qt, h, ci, c0, c1, nch):
                """One pipeline unit: QK scores + per-chunk max for chunk ci;
                on the last chunk, final max + all exps + transpose."""
                st = get_qt_tiles(qt)
                klen = (qt + 1) * P
                qtl = qsb[h // 2][:, qt * P:(qt + 1) * P]
                hs = hstate.setdefault((qt, h), {"scs": [], "mt": None, "pc": None})
                if ci == 0:
                    hs["pc"] = ppool.tile([P, klen], f16, tag="p",
                                          padded_shape=[P, S], name="pc")
                    hs["mt"] = stat.tile([P, 2], f32, tag="mt", name="mt")
                mt, pc = hs["mt"], hs["pc"]
                cl = c1 - c0
                sc = scpool.tile([P, CHUNK], f32, tag="scores", name="sc")
                hs["scs"].append(sc)
                if c1 == klen:
                    # final chunk: fold the causal mask into the PSUM
                    # accumulation of the last sub-chunk via ident^T@maskw
                    # (sub-chunks stay 512-aligned: PSUM bank boundaries)
                    last = (cl - 1) // 512 * 512
                    for n0 in range(0, last, 512):
                        nc.tensor.matmul(sc[:, n0:n0 + 512], qtl,
                                         kdup[h][:, c0 + n0:c0 + n0 + 512],
                                         start=True, stop=True)
                    nn = cl - last
                    nc.tensor.matmul(sc[:, last:cl], qtl,
                                     kdup[h][:, c0 + last:c0 + cl],
                                     start=True, stop=False)
                    nc.tensor.matmul(sc[:, last:cl], ident_sb[:],
                                     maskw_sb[:, 512 - nn:512],
                                     start=False, stop=True)
                else:
                    for n0 in range(0, cl, 512):
                        nn = min(512, cl - n0)
                        nc.tensor.matmul(sc[:, n0:n0 + nn], qtl,
                                         kdup[h][:, c0 + n0:c0 + n0 + nn],
                                         start=True, stop=True)
                if nch == 1:
                    nc.vector.reduce_max(mt[:, 0:1], sc[:, :cl], axis=AX, negate=True)
                else:
                    nc.vector.reduce_max(mt[:, ci:ci + 1], sc[:, :cl], axis=AX,
                                         negate=False)
                if c1 != klen:
                    return
                # last chunk of (qt, h): final max, exps, transpose
                if nch == 1:
                    mf = mt[:, 0:1]
                else:
                    mf = stat.tile([P, 1], f32, tag="mf", name="mf")
                    nc.vector.reduce_max(mf, mt[:, 0:nch], axis=AX, negate=True)
                chunks = chunks_of(qt)
                for cj, (d0, d1) in enumerate(chunks):
                    nc.scalar.activation(pc[:, d0:d1], hs["scs"][cj][:, :d1 - d0],
                                         AF.Exp, bias=mf, scale=1.0)
                pt = ptpool.tile([P, QT, P], f16, tag="pt", name="pt")
                st["pt"][h] = pt
                nc.sync.dma_start_transpose(pt[:, 0:klen // P, :], pc[:])
                del hstate[(qt, h)]

            def beta(qt, h):
                st = get_qt_tiles(qt)
                pt = st["pt"][h]
                nkb = qt + 1
                # PV over all kblocks; vsb column DK4 (ones) makes col 64 = Z
                ops = pvpool.tile([P, DK + 1], f32, tag="pv", name="pvt")
                for kb in range(nkb):
                    nc.tensor.matmul(
                        ops[:], pt[:, kb, :],
                        vsb[kb][:, h, :],
                        start=(kb == 0), stop=(kb == nkb - 1))
                rh = stat.tile([P, 1], f32, tag="rh")
                nc.vector.reciprocal(rh, ops[:, DK:DK + 1])
                nc.scalar.activation(
                    st["ostg"][:, h * DK:(h + 1) * DK], ops[:, 0:DK], AF.Copy, scale=rh)

            def out_proj(qt):
                for nn2 in range(2):
                    yt = scpool.tile([P, CHUNK], f32, tag="scores", name="yps")
                    yps = yt[:, 0:512]
                    for j in range(NT):
                        nc.tensor.matmul(
                            yps, outT[:, j, qt * P:(qt + 1) * P],
                            wo_sb[:, j, nn2 * 512:(nn2 + 1) * 512],
                            start=(j == 0), stop=(j == NT - 1))
                    ysb = ypool.tile([P, 512], f32, tag="y", name="ysb")
                    nc.vector.tensor_copy(ysb[:], yps)
                    nc.gpsimd.dma_start(
                        out=y[qt * P:(qt + 1) * P, nn2 * 512:(nn2 + 1) * 512], in_=ysb[:])

            def finish_qt(qt):
                st = state[qt]
                nc.sync.dma_start_transpose(outT[:, :, qt * P:(qt + 1) * P], st["ostg"][:])
                del state[qt]["pt"]

            ready = []   # (qt, h) pairs whose alpha is fully emitted
            bidx = 0
            for qt, h, ci, c0, c1, nch in units:
                alpha_chunk(qt, h, ci, c0, c1, nch)
                if c1 == (qt + 1) * P:
                    ready.append((qt, h))
                while len(ready) - bidx > 2:
                    bqt, bh = ready[bidx]
                    bidx += 1
                    beta(bqt, bh)
                    if bh == HL - 1:
                        finish_qt(bqt)
            for bqt, bh in ready[bidx:]:
                beta(bqt, bh)
                if bh == HL - 1:
                    finish_qt(bqt)
            for qt in range(QT):
                out_proj(qt)


def _host_prep(q, k, v, wq, wk, wv, wo):
    """Build the 8 per-core input maps."""
    ident = np.eye(P, dtype=np.float32)
    maskw = np.zeros((P, 512), np.float32)
    maskw[:, 384:512] = np.triu(np.full((P, P), -BIG, np.float32), k=1)
    in_maps = []
    per_b = {}
    for b in range(B):
        per_b[b] = (
            np.ascontiguousarray(q[b].T.astype(np.float32)),
            np.ascontiguousarray(k[b].T.astype(np.float32)),
            np.ascontiguousarray(v[b].T.astype(np.float32)).astype(np.float16),
        )
    per_g = {}
    for g in range(2):
        cs = slice(g * DL, (g + 1) * DL)
        per_g[g] = (
            np.ascontiguousarray(wq[:, cs].astype(np.float32)),
            np.ascontiguousarray(wk[:, cs].astype(np.float32)),
            np.ascontiguousarray(wv[:, cs]).astype(np.float16),
            np.ascontiguousarray(wo[cs, :]).astype(np.float16),
        )
    for c in range(N_CORES):
        b, g = c // 2, c % 2
        xq_c, xk_c, xv_c = per_b[b]
        wq_c, wk_c, wv_c, wo_c = per_g[g]
        in_maps.append({
            "xq": xq_c, "xk": xk_c, "xv": xv_c,
            "wq": wq_c, "wk": wk_c, "wv": wv_c, "wo": wo_c,
            "ident": ident, "maskw": maskw,
        })
    return in_maps


def kernel(q, k, v, wq, wk, wv, wo):
    if "nc" not in _cache:
        _cache["nc"] = _build()
    nc = _cache["nc"]
    in_maps = _host_prep(np.asarray(q), np.asarray(k), np.asarray(v),
                         np.asarray(wq), np.asarray(wk), np.asarray(wv),
                         np.asarray(wo))
    res = run_bass_kernel_spmd(nc, in_maps, list(range(N_CORES)))
    out = np.empty((B, S, D), np.float32)
    for b in range(B):
        out[b] = res.results[2 * b]["y"] + res.results[2 * b + 1]["y"]
    return out


if __name__ == "__main__":
    d = np.load("/root/problem/inputs_cache.npz")
    out = kernel(d["q"], d["k"], d["v"], d["wq"], d["wk"], d["wv"], d["wo"])
    ref = d["ref"]
    rel = np.linalg.norm(out - ref) / np.linalg.norm(ref)
    print(f"Relative error: {rel:.4e}")

